# revision 1
# baseline (speedup 1.0000x reference)
"""FFTConv1d-equivalent direct convolution kernel for 8x TRN2 NeuronCores.

out[b,o,l] = sum_{i,k} x_pad[b,i,l+k] * w[o,i,k] + bias[o]   (cross-correlation,
'same' padding PAD_L=2047/PAD_R=2048 — matches the FFT reference exactly).

Sharding: 2 out-channel halves x 4 L-quarters = 8 cores. Each core computes
out[:, og*128:(og+1)*128, lg*2048:(lg+1)*2048] with full-128-partition matmuls.

Per core: k-loop accumulation in PSUM, bf16 operands, fp32 accumulate.
Weights are streamed from DRAM in 64-tap blocks via a hardware For_i loop
(dynamic DMA offset), double-buffered.
"""
import os
import sys

sys.path.insert(0, "/opt/trn_rl_repo")

import numpy as np
import ml_dtypes

B, C_IN, C_OUT, L, K = 8, 256, 256, 8192, 4096
PAD_L, PAD_R = 2047, 2048
N_CORES = 8
OG, LG = 2, 4            # out-channel halves x L quarters
O_SH = C_OUT // OG       # 128 out channels per core
L_SH = L // LG           # 2048 output cols per core
XCOLS = L_SH + K         # 6143 -> pad to 6144 local x cols per core
XC = 6144
IC = 2                   # input-channel chunks of 128
KB = 32                  # taps per weight block
NKB = K // KB            # 64 blocks
BF16 = ml_dtypes.bfloat16

_CACHE = {}


def _build():
    import concourse.tile as tile
    from concourse import bacc, mybir
    import concourse.bass as bass

    nc = bacc.Bacc("TRN2", target_bir_lowering=False, debug=False,
                   num_devices=N_CORES)
    # weights pre-arranged on host: rows = (ic, kb, i) blocks, cols = (kin, o)
    wd = nc.dram_tensor("wd", [IC * NKB * 128, KB * O_SH], mybir.dt.bfloat16,
                        kind="ExternalInput").ap()
    xd = nc.dram_tensor("xd", [B, IC, 128, XC], mybir.dt.bfloat16,
                        kind="ExternalInput").ap()
    bd = nc.dram_tensor("bd", [O_SH, 1], mybir.dt.float32,
                        kind="ExternalInput").ap()
    od = nc.dram_tensor("od", [B, O_SH, L_SH], mybir.dt.float32,
                        kind="ExternalOutput").ap()

    with tile.TileContext(nc) as tc:
        with tc.tile_pool(name="xp", bufs=1) as xp, \
             tc.tile_pool(name="wp", bufs=2) as wp, \
             tc.tile_pool(name="cst", bufs=1) as cst, \
             tc.tile_pool(name="outp", bufs=4) as outp, \
             tc.tile_pool(name="ps", bufs=8, space="PSUM") as ps:

            zt = cst.tile([1, O_SH], mybir.dt.bfloat16)
            nc.vector.memset(zt[:], 0.0)
            bias_sb = cst.tile([O_SH, 1], mybir.dt.float32)
            nc.sync.dma_start(bias_sb[:], bd)

            # groups: (b-half, l-pair) -> 8 psum tiles each
            for bh in range(2):
                for ltp in range(2):
                    # load x tiles for this group's 4 batches
                    xs = {}
                    for bl in range(4):
                        b = bh * 4 + bl
                        for ic in range(IC):
                            t = xp.tile([128, XC], mybir.dt.bfloat16,
                                        tag=f"x{bl}_{ic}")
                            nc.sync.dma_start(t[:], xd[b, ic, :, :])
                            xs[(bl, ic)] = t

                    pts = []
                    for bl in range(4):
                        for tl in range(2):
                            pt = ps.tile([O_SH, 512], mybir.dt.float32)
                            # dummy zero matmul to open the accum group
                            nc.tensor.matmul(
                                pt[:], zt[0:1, :], xs[(bl, 0)][0:1, 0:512],
                                start=True, stop=False, skip_group_check=True)
                            pts.append((bl, tl, pt))

                    for ic in range(IC):
                        with tc.For_i(0, NKB, 1) as kb:
                            wt = wp.tile([128, KB * O_SH], mybir.dt.bfloat16,
                                         tag="w")
                            roff = (ic * NKB) * 128
                            nc.sync.dma_start(
                                wt[:], wd[bass.ds(kb * 128 + roff, 128), :])
                            kbase = kb * KB
                            for kin in range(KB):
                                lhs = wt[:, kin * O_SH:(kin + 1) * O_SH]
                                for bl, tl, pt in pts:
                                    off = kin + ltp * 1024 + tl * 512
                                    rhs = xs[(bl, ic)][:, bass.ds(kbase + off,
                                                                  512)]
                                    nc.tensor.matmul(
                                        pt[:], lhs, rhs, start=False,
                                        stop=False, skip_group_check=True)

                    # close groups, add bias, write out
                    for bl, tl, pt in pts:
                        b = bh * 4 + bl
                        nc.tensor.matmul(
                            pt[:], zt[0:1, :], xs[(bl, 0)][0:1, 0:512],
                            start=False, stop=True, skip_group_check=True)
                        ot = outp.tile([O_SH, 512], mybir.dt.float32, tag="o")
                        nc.scalar.add(ot[:], pt[:], bias_sb[:, 0:1])
                        nc.sync.dma_start(
                            od[b, :, ltp * 1024 + tl * 512:
                               ltp * 1024 + tl * 512 + 512], ot[:])
    nc.compile()
    return nc


def kernel(x, weight, bias):
    from concourse import bass_utils

    if "nc" not in _CACHE:
        _CACHE["nc"] = _build()
    nc = _CACHE["nc"]

    xpad = np.zeros((B, C_IN, PAD_L + L + PAD_R + 1), dtype=np.float32)
    xpad[:, :, PAD_L:PAD_L + L] = x
    in_maps = []
    for g in range(N_CORES):
        og, lg = g // LG, g % LG
        xs = xpad[:, :, lg * L_SH: lg * L_SH + XC]           # [B, 256, 6144]
        xs = np.ascontiguousarray(xs).astype(BF16)
        xs = xs.reshape(B, IC, 128, XC)
        w = weight[og * O_SH:(og + 1) * O_SH]                # [128, 256, 4096]
        # -> [ic, kb, i, kin, o]
        wp = w.reshape(O_SH, IC, 128, NKB, KB).transpose(1, 3, 2, 4, 0)
        wp = np.ascontiguousarray(wp).astype(BF16)
        wp = wp.reshape(IC * NKB * 128, KB * O_SH)
        bs = bias[og * O_SH:(og + 1) * O_SH].reshape(O_SH, 1)
        in_maps.append({"wd": wp, "xd": xs,
                        "bd": np.ascontiguousarray(bs, dtype=np.float32)})

    trace = bool(int(os.environ.get("BASS_CONV_TRACE", "0")))
    res = bass_utils.run_bass_kernel_spmd(
        nc, in_maps, core_ids=list(range(N_CORES)), trace=trace)
    _CACHE["last_result"] = res

    out = np.empty((B, C_OUT, L), dtype=np.float32)
    for g in range(N_CORES):
        og, lg = g // LG, g % LG
        out[:, og * O_SH:(og + 1) * O_SH, lg * L_SH:(lg + 1) * L_SH] = \
            res.results[g]["od"]
    return out



# revision 2
# speedup vs baseline: 1.1064x; 1.1064x over previous
"""FFTConv1d-equivalent direct convolution kernel for 8x TRN2 NeuronCores.

out[b,o,l] = sum_{i,k} x_pad[b,i,l+k] * w[o,i,k] + bias[o]   (cross-correlation,
'same' padding PAD_L=2047/PAD_R=2048 — matches the FFT reference exactly).

Sharding: 2 out-channel halves x 4 L-quarters = 8 cores. Each core computes
out[:, og*128:(og+1)*128, lg*2048:(lg+1)*2048] with full-128-partition matmuls.

Per core: k-loop accumulation in PSUM, bf16 operands, fp32 accumulate.
Weight streaming is software-pipelined inside the hardware loop: each body
processes 4 tap-blocks from two fixed SBUF buffers (A/B), prefetching the
next block during the other buffer's compute. One LDWEIGHTS per tap feeds
8 matmuls (ldweights=False) that share the stationary operand.
"""
import os
import sys

sys.path.insert(0, "/opt/trn_rl_repo")

import numpy as np
import ml_dtypes

B, C_IN, C_OUT, L, K = 8, 256, 256, 8192, 4096
PAD_L, PAD_R = 2047, 2048
N_CORES = 8
OG, LG = 2, 4            # out-channel halves x L quarters
O_SH = C_OUT // OG       # 128 out channels per core
L_SH = L // LG           # 2048 output cols per core
XCOLS = L_SH + K         # 6143 -> pad to 6144 local x cols per core
XC = 6144
IC = 2                   # input-channel chunks of 128
KB = 32                  # taps per weight block
NKB = K // KB            # 128 blocks
BPB = 4                  # blocks per hardware-loop body
BF16 = ml_dtypes.bfloat16
LDW_DEDUP = bool(int(os.environ.get("BASS_CONV_LDW_DEDUP", "1")))

_CACHE = {}


def _build():
    import concourse.tile as tile
    from concourse import bacc, mybir
    import concourse.bass as bass

    nc = bacc.Bacc("TRN2", target_bir_lowering=False, debug=False,
                   num_devices=N_CORES)
    # weights pre-arranged on host: rows = (ic, kb, i) blocks, cols = (kin, o)
    # one extra zero block at the end so the last in-loop prefetch stays
    # in bounds
    wd = nc.dram_tensor("wd", [(IC * NKB + 1) * 128, KB * O_SH],
                        mybir.dt.bfloat16, kind="ExternalInput").ap()
    xd = nc.dram_tensor("xd", [B, IC, 128, XC], mybir.dt.bfloat16,
                        kind="ExternalInput").ap()
    bd = nc.dram_tensor("bd", [O_SH, 1], mybir.dt.float32,
                        kind="ExternalInput").ap()
    od = nc.dram_tensor("od", [B, O_SH, L_SH], mybir.dt.float32,
                        kind="ExternalOutput").ap()

    with tile.TileContext(nc) as tc:
        with tc.tile_pool(name="xp", bufs=1) as xp, \
             tc.tile_pool(name="wp", bufs=1) as wp, \
             tc.tile_pool(name="cst", bufs=1) as cst, \
             tc.tile_pool(name="outp", bufs=4) as outp, \
             tc.tile_pool(name="ps", bufs=8, space="PSUM") as ps:

            zt = cst.tile([1, O_SH], mybir.dt.bfloat16)
            nc.vector.memset(zt[:], 0.0)
            bias_sb = cst.tile([O_SH, 1], mybir.dt.float32)
            nc.sync.dma_start(bias_sb[:], bd)

            # groups: (b-half, l-pair) -> 8 psum tiles each
            for bh in range(2):
                for ltp in range(2):
                    # load x tiles for this group's 4 batches
                    xs = {}
                    for bl in range(4):
                        b = bh * 4 + bl
                        for ic in range(IC):
                            t = xp.tile([128, XC], mybir.dt.bfloat16,
                                        tag=f"x{bl}_{ic}")
                            nc.sync.dma_start(t[:], xd[b, ic, :, :])
                            xs[(bl, ic)] = t

                    pts = []
                    for bl in range(4):
                        for tl in range(2):
                            pt = ps.tile([O_SH, 512], mybir.dt.float32)
                            # dummy zero matmul to open the accum group
                            nc.tensor.matmul(
                                pt[:], zt[0:1, :], xs[(bl, 0)][0:1, 0:512],
                                start=True, stop=False, skip_group_check=True)
                            pts.append((bl, tl, pt))

                    def compute(wt, it, ic, j):
                        # block index = it*BPB + j; taps [blk*KB, blk*KB+KB)
                        for kin in range(KB):
                            lhs = wt[:, kin * O_SH:(kin + 1) * O_SH]
                            if LDW_DEDUP:
                                nc.tensor.ldweights(lhs)
                            for bl, tl, pt in pts:
                                off = (j * KB + kin + ltp * 1024 + tl * 512)
                                rhs = xs[(bl, ic)][:, bass.ds(
                                    it * (BPB * KB) + off, 512)]
                                inst = nc.tensor.matmul(
                                    pt[:], lhs, rhs, start=False,
                                    stop=False, skip_group_check=True)
                                if LDW_DEDUP:
                                    inst.ldweights = False

                    for ic in range(IC):
                        roff = (ic * NKB) * 128
                        wA = wp.tile([128, KB * O_SH], mybir.dt.bfloat16,
                                     tag="wA")
                        wB = wp.tile([128, KB * O_SH], mybir.dt.bfloat16,
                                     tag="wB")
                        # preamble: block 0 of this ic chunk into A
                        nc.sync.dma_start(wA[:], wd[bass.ds(roff, 128), :])
                        with tc.For_i(0, NKB // BPB, 1) as it:
                            rbase = it * (BPB * 128) + roff
                            for j in range(BPB):
                                buf, nbuf = (wA, wB) if j % 2 == 0 else (wB, wA)
                                # prefetch block it*BPB + j + 1 into the
                                # other buffer
                                nc.sync.dma_start(
                                    nbuf[:],
                                    wd[bass.ds(rbase + (j + 1) * 128, 128), :])
                                compute(buf, it, ic, j)

                    # close groups, add bias, write out
                    for bl, tl, pt in pts:
                        b = bh * 4 + bl
                        nc.tensor.matmul(
                            pt[:], zt[0:1, :], xs[(bl, 0)][0:1, 0:512],
                            start=False, stop=True, skip_group_check=True)
                        ot = outp.tile([O_SH, 512], mybir.dt.float32, tag="o")
                        nc.scalar.add(ot[:], pt[:], bias_sb[:, 0:1])
                        nc.sync.dma_start(
                            od[b, :, ltp * 1024 + tl * 512:
                               ltp * 1024 + tl * 512 + 512], ot[:])
    nc.compile()
    return nc


def kernel(x, weight, bias):
    from concourse import bass_utils

    if "nc" not in _CACHE:
        _CACHE["nc"] = _build()
    nc = _CACHE["nc"]

    xpad = np.zeros((B, C_IN, PAD_L + L + PAD_R + 1), dtype=np.float32)
    xpad[:, :, PAD_L:PAD_L + L] = x
    in_maps = []
    for g in range(N_CORES):
        og, lg = g // LG, g % LG
        xs = xpad[:, :, lg * L_SH: lg * L_SH + XC]           # [B, 256, 6144]
        xs = np.ascontiguousarray(xs).astype(BF16)
        xs = xs.reshape(B, IC, 128, XC)
        w = weight[og * O_SH:(og + 1) * O_SH]                # [128, 256, 4096]
        # -> [ic, kb, i, kin, o]
        wp = w.reshape(O_SH, IC, 128, NKB, KB).transpose(1, 3, 2, 4, 0)
        wp = np.ascontiguousarray(wp).astype(BF16)
        wp = wp.reshape(IC * NKB * 128, KB * O_SH)
        wp = np.concatenate(
            [wp, np.zeros((128, KB * O_SH), dtype=BF16)], axis=0)
        bs = bias[og * O_SH:(og + 1) * O_SH].reshape(O_SH, 1)
        in_maps.append({"wd": wp, "xd": xs,
                        "bd": np.ascontiguousarray(bs, dtype=np.float32)})

    trace = bool(int(os.environ.get("BASS_CONV_TRACE", "0")))
    res = bass_utils.run_bass_kernel_spmd(
        nc, in_maps, core_ids=list(range(N_CORES)), trace=trace)
    _CACHE["last_result"] = res

    out = np.empty((B, C_OUT, L), dtype=np.float32)
    for g in range(N_CORES):
        og, lg = g // LG, g % LG
        out[:, og * O_SH:(og + 1) * O_SH, lg * L_SH:(lg + 1) * L_SH] = \
            res.results[g]["od"]
    return out
